# revision 14
# baseline (speedup 1.0000x reference)
"""BarCachedCrossAttention Trainium2 kernel (v5).

Sharding: 8 cores = 4 batches x 2 head-groups (8 heads / 512 channels each).
Per core, everything is computed in a transposed layout (partition = context
token for scores) so probs never need a transpose: U^T = V'^T @ P^T with a
ones-column in V' producing the softmax denominators for free, and the
instrument mask is applied by zeroing masked tokens' V' rows.

Key implementation notes:
  - all matmul operands are fp16 (1 cyc/row on the PE; f32 PSUM accum),
    probs are fp16 with a constant exp shift of -14 (cancels in U/Z),
  - embeddings are pre-added into context on the host; K-bias cancels in
    softmax; V-bias folds into the output bias (bv @ Wo.T) on the host,
  - inputs stream over three DMA queues (sync + scalar HWDGE, gpsimd
    SWDGE); Q-path arrives in two half-chunks so the PE starts early,
  - K^T / V' tiles are persistent (all 16 token tiles) so slab ns+1's
    projections overlap slab ns's attention without WAR stalls,
  - one 2048-wide exp ACT per j-iteration ([128, hi, half, 512] PSUM tile),
  - softmax normalization is fused into the last slab: 1/Z broadcast uses a
    DMA partition_broadcast (no PSUM) and the scale runs on idle GpSimd,
  - reciprocal_approx_fast requires a base-partition-0 operand: Z must be
    copied out of U's partition 64 first (hw misreads it otherwise).
"""

import sys

sys.path.insert(0, "/opt/trn_rl_repo")

import numpy as np

import concourse.bacc as bacc
import concourse.tile as tile
from concourse import mybir
from concourse.bass_utils import run_bass_kernel_spmd

B, T, N_CTX, H = 4, 512, 2048, 1024
NUM_HEADS, NUM_INSTRUMENTS, MAX_BARS = 16, 16, 8
HEAD_DIM = H // NUM_HEADS  # 64
HG = 2  # head groups (cores per batch)
CH = H // HG  # 512 channels per core
NH_G = NUM_HEADS // HG  # 8 heads per core
P = 128
F32 = mybir.dt.float32
FP16 = mybir.dt.float16
DT = FP16  # all matmul operands
SHIFT = -14.0  # constant exp shift: scores max ~22 -> exp <= e^8 fits fp16

KC = H // P  # 8 contraction chunks for K/V/Q projections
PT_CH = CH // P  # 4 partition tiles of channels
NS = N_CTX // 512  # 4 context slabs of 512 tokens
NT = N_CTX // P  # 16 context tiles of 128 tokens
TT = T // P  # 4 tiles of query tokens

_compiled = None


def _build():
    nc = bacc.Bacc("TRN2", target_bir_lowering=False, debug=False, num_devices=8)

    qT_d = nc.dram_tensor("qT", [2, KC // 2, P, T], DT, kind="ExternalInput")
    wq_d = nc.dram_tensor("wqT", [2, KC // 2, P, CH], DT, kind="ExternalInput")
    wk_d = nc.dram_tensor("wkT", [P, KC, CH], DT, kind="ExternalInput")
    wv_d = nc.dram_tensor("wvT", [P, KC, CH], DT, kind="ExternalInput")
    ctx_d = nc.dram_tensor("ctxT", [NS, P, KC, 512], DT, kind="ExternalInput")
    wo_d = nc.dram_tensor("woT", [P, PT_CH, H], DT, kind="ExternalInput")
    mb_d = nc.dram_tensor("mb", [P, NT], F32, kind="ExternalInput")
    bqe_d = nc.dram_tensor("bqe", [P, PT_CH], F32, kind="ExternalInput")
    out_d = nc.dram_tensor("out", [T, H], F32, kind="ExternalOutput")

    with tile.TileContext(nc) as tc:
        with (
            nc.allow_low_precision(reason="16-bit matmul operands; accum stays f32"),
            tc.tile_pool(name="persist", bufs=1) as pers,
        ):
            # Q-path inputs on the sync HWDGE queue in two 512KB half-chunks
            bqe = pers.tile([P, PT_CH], F32, name="bqe")
            nc.sync.dma_start(bqe[:], bqe_d.ap())
            qt = pers.tile([P, 2, KC // 2, T], DT, name="qt_in")
            wq = pers.tile([P, 2, KC // 2, CH], DT, name="wq")
            for c in range(2):
                nc.sync.dma_start(
                    qt[:, c, :, :],
                    qT_d.ap()[c].rearrange("k p t -> p k t"))
                nc.sync.dma_start(
                    wq[:, c, :, :],
                    wq_d.ap()[c].rearrange("k p t -> p k t"))
            # K/V/O weights on the scalar HWDGE queue (parallel with sync)
            wk = pers.tile([P, KC, CH], DT, name="wk")
            nc.scalar.dma_start(wk[:], wk_d.ap())
            wv = pers.tile([P, KC, CH], DT, name="wv")
            nc.scalar.dma_start(wv[:], wv_d.ap())
            wo = pers.tile([P, PT_CH, H], DT, name="wo")
            nc.scalar.dma_start(wo[:], wo_d.ap())
            # mask + context slabs on the gpsimd SWDGE queue
            mb = pers.tile([P, NT], F32, name="mb")
            nc.gpsimd.dma_start(mb[:], mb_d.ap())

            ones8 = pers.tile([P, NH_G], F32, name="ones8")
            nc.vector.memset(ones8[:], 1.0)
            shiftb = pers.tile([P, 1], F32, name="shiftb")
            nc.vector.memset(shiftb[:], SHIFT)

            QT = [pers.tile([P, T], DT, name=f"qt{p}") for p in range(PT_CH)]
            OT = [pers.tile([P, T], DT, name=f"ot{p}") for p in range(PT_CH)]
            U = [pers.tile([HEAD_DIM + 1, T], F32, name=f"u{h}") for h in range(NH_G)]
            # persistent K^T / V' tiles for all 16 token tiles
            KT = [pers.tile([P, 512], DT, name=f"kt{i}") for i in range(NS * PT_CH)]
            VT = [pers.tile([P, NH_G, HEAD_DIM + 1], DT, name=f"vt{i}") for i in range(NT)]

            # ---- Q projection: consumes the two DMA half-chunks in order ----
            with tc.tile_pool(name="qps", bufs=1, space="PSUM") as qps:
                ps_q = [qps.tile([P, 512], F32, name=f"ps_q{p}") for p in range(PT_CH)]
                for c in range(2):
                    for kk in range(KC // 2):
                        for p in range(PT_CH):
                            nc.tensor.matmul(
                                ps_q[p][:],
                                wq[:, c, kk, p * P : (p + 1) * P],
                                qt[:, c, kk, :],
                                start=(c == 0 and kk == 0),
                                stop=(c == 1 and kk == KC // 2 - 1),
                            )
                for p in range(PT_CH):
                    nc.vector.tensor_scalar_add(QT[p][:], ps_q[p][:], bqe[:, p : p + 1])

            # ---- fused K/V projection + attention, one 512-token slab at a time ----
            with (
                tc.tile_pool(name="slab", bufs=3) as slabp,
                tc.tile_pool(name="ptp", bufs=2) as ptp,
                tc.tile_pool(name="usb", bufs=2) as usb,
                tc.tile_pool(name="kvps", bufs=2, space="PSUM") as kvps,
                tc.tile_pool(name="sps", bufs=1, space="PSUM") as sps,
                tc.tile_pool(name="ups", bufs=2, space="PSUM") as ups,
            ):
                for ns in range(NS):
                    slab = slabp.tile([P, KC, 512], DT, name="slab")
                    nc.gpsimd.dma_start(slab[:], ctx_d.ap()[ns])
                    # K^T columns for this slab: 4 partition tiles of channels
                    for p in range(PT_CH):
                        ps = kvps.tile([P, 512], F32, name="ps_kv")
                        for k in range(KC):
                            nc.tensor.matmul(
                                ps[:],
                                wk[:, k, p * P : (p + 1) * P],
                                slab[:, k, :],
                                start=(k == 0), stop=(k == KC - 1),
                            )
                        nc.vector.tensor_copy(KT[ns * PT_CH + p][:], ps[:])
                    # V' tiles (with masked rows zeroed, ones column for Z)
                    for s4 in range(4):
                        i = ns * 4 + s4
                        psv = kvps.tile([P, 512], F32, name="ps_kv")
                        for k in range(KC):
                            nc.tensor.matmul(
                                psv[:],
                                slab[:, k, s4 * P : (s4 + 1) * P],
                                wv[:, k, :],
                                start=(k == 0), stop=(k == KC - 1),
                            )
                        nc.vector.tensor_scalar_mul(
                            VT[i][:, :, :HEAD_DIM],
                            psv[:].rearrange("p (h d) -> p h d", d=HEAD_DIM),
                            mb[:, i : i + 1],
                        )
                        nc.vector.tensor_scalar_mul(
                            VT[i][:, :, HEAD_DIM], ones8[:], mb[:, i : i + 1]
                        )
                    # attention: scores (hi pair on disjoint 64-row PE groups)
                    # -> one 2048-wide exp -> U accumulation
                    for hp in range(NH_G // 2):
                        p = hp
                        kt = KT[ns * PT_CH + p]
                        psus = [ups.tile([HEAD_DIM + 1, 512], F32, name="ps_u") for _ in range(2)]
                        for j in range(2):
                            pss = sps.tile([P, 2, 2, 512], F32, name="ps_s")
                            pts = ptp.tile([P, 2, 2, 512], DT, name="pt")
                            for half in range(2):
                                s4 = 2 * j + half
                                for hi in range(2):
                                    d0, d1 = hi * HEAD_DIM, (hi + 1) * HEAD_DIM
                                    nc.tensor.matmul(
                                        pss[:, hi, half, :],
                                        kt[d0:d1, s4 * P : (s4 + 1) * P],
                                        QT[p][d0:d1, :],
                                        start=True, stop=True,
                                    )
                            nc.scalar.activation(
                                pts[:], pss[:], mybir.ActivationFunctionType.Exp,
                                bias=shiftb[:], scale=0.125,
                            )
                            for hi in range(2):
                                for half in range(2):
                                    s4 = 2 * j + half
                                    nc.tensor.matmul(
                                        psus[hi][:], VT[ns * 4 + s4][:, 2 * hp + hi, :],
                                        pts[:, hi, half, :],
                                        start=(j == 0 and half == 0),
                                        stop=(j == 1 and half == 1),
                                    )
                        for hi in range(2):
                            h = 2 * hp + hi
                            if ns == 0:
                                nc.vector.tensor_copy(U[h][:], psus[hi][:])
                            else:
                                nc.vector.tensor_add(U[h][:], U[h][:], psus[hi][:])
                        if ns == NS - 1:
                            # normalization fused into the last slab: DMA
                            # partition-broadcast of 1/Z, scale on GpSimd
                            for hi in range(2):
                                h = 2 * hp + hi
                                zst = usb.tile([1, T], F32, name="zst")
                                nc.vector.tensor_copy(
                                    zst[:], U[h][HEAD_DIM : HEAD_DIM + 1, :])
                                rf = usb.tile([1, T], F32, name="rf")
                                nc.vector.reciprocal_approx_fast(rf[:], zst[:])
                                rb = usb.tile([HEAD_DIM, T], F32, name="rb")
                                nc.gpsimd.partition_broadcast(
                                    rb[:], rf[:], channels=HEAD_DIM)
                                d0, d1 = hi * HEAD_DIM, (hi + 1) * HEAD_DIM
                                nc.gpsimd.tensor_tensor(
                                    OT[p][d0:d1, :], U[h][:HEAD_DIM, :], rb[:],
                                    op=mybir.AluOpType.mult,
                                )

            # ---- output projection ----
            with (
                tc.tile_pool(name="ob", bufs=3) as obp,
                tc.tile_pool(name="ops", bufs=3, space="PSUM") as ops,
            ):
                for tt in range(TT):
                    for o in range(2):
                        pso = ops.tile([P, 512], F32, name="ps_o")
                        for p in range(PT_CH):
                            nc.tensor.matmul(
                                pso[:],
                                OT[p][:, tt * P : (tt + 1) * P],
                                wo[:, p, o * 512 : (o + 1) * 512],
                                start=(p == 0), stop=(p == PT_CH - 1),
                            )
                        ob = obp.tile([P, 512], F32, name="ob")
                        nc.scalar.copy(ob[:], pso[:])
                        nc.sync.dma_start(
                            out_d.ap()[tt * P : (tt + 1) * P, o * 512 : (o + 1) * 512],
                            ob[:],
                        )

    nc.compile()
    return nc


def _prep_inputs(query, context, instrument_ids, current_instrument_id, bar_offsets,
                 Wq, bq, Wk, bk, Wv, bv, Wo, bo, inst_emb, bar_emb):
    f32, f16 = np.float32, np.float16
    query = np.asarray(query, f32)
    context = np.asarray(context, f32)
    inst = np.asarray(instrument_ids).astype(np.int64)
    bars = np.clip(np.asarray(bar_offsets).astype(np.int64), 0, MAX_BARS - 1)
    cur = min(max(int(np.asarray(current_instrument_id)), 0), NUM_INSTRUMENTS - 1)
    Wq, Wk, Wv, Wo = (np.asarray(w, f32) for w in (Wq, Wk, Wv, Wo))
    bq, bv, bo = (np.asarray(b, f32) for b in (bq, bv, bo))
    inst_emb = np.asarray(inst_emb, f32)
    bar_emb = np.asarray(bar_emb, f32)

    ctx = context + inst_emb[inst] + bar_emb[bars]        # (B,N,H) host pre-add
    qh = query + inst_emb[cur][None, None, :]
    bq_eff = bq + 0.0

    def perm(x):  # (H, F) -> [128, KC, F] contiguous
        Hd, F = x.shape
        return np.ascontiguousarray(x.reshape(KC, P, F).transpose(1, 0, 2).astype(f16))

    def perm_k2(x):  # (H, F) -> [2, KC//2, 128, F] contiguous half-chunks
        Hd, F = x.shape
        return np.ascontiguousarray(x.reshape(2, KC // 2, P, F).astype(f16))

    WqT, WkT, WvT, WoT = Wq.T, Wk.T, Wv.T, Wo.T

    in_maps = []
    for b in range(B):
        qT = perm_k2(np.ascontiguousarray(qh[b].T))       # [2, 4, 128, 512]
        ctxT = ctx[b].T                                   # (H, N)
        # [NS, 128, KC, 512]
        ctx_p = np.ascontiguousarray(
            ctxT.reshape(KC, P, NS, 512).transpose(2, 1, 0, 3).astype(f16))
        mbv = np.where(inst[b] == cur, 0.0, 1.0).astype(f32)
        mbt = np.ascontiguousarray(mbv.reshape(NT, P).T)  # (128, NT)
        for g in range(HG):
            sl = slice(g * CH, (g + 1) * CH)
            woT = np.ascontiguousarray(
                WoT[sl, :].reshape(PT_CH, P, H).transpose(1, 0, 2).astype(f16))
            in_maps.append({
                "qT": qT,
                "ctxT": ctx_p,
                "wqT": perm_k2(WqT[:, sl]),
                "wkT": perm(WkT[:, sl]),
                "wvT": perm(WvT[:, sl]),
                "woT": woT,
                "mb": mbt,
                "bqe": np.ascontiguousarray(bq_eff[sl].reshape(PT_CH, P).T),
            })
    return in_maps, bo + bv @ Wo.T


def kernel(**inputs) -> np.ndarray:
    global _compiled
    if _compiled is None:
        _compiled = _build()
    in_maps, bo_eff = _prep_inputs(**inputs)
    res = run_bass_kernel_spmd(_compiled, in_maps, list(range(B * HG))).results
    out = np.empty((B, T, H), np.float32)
    for b in range(B):
        out[b] = res[b * HG]["out"] + res[b * HG + 1]["out"] + bo_eff
    return out


# revision 15
# speedup vs baseline: 1.4396x; 1.4396x over previous
"""BarCachedCrossAttention Trainium2 kernel (v6).

Sharding: 8 cores = 4 batches x 2 head-groups (8 heads / 512 channels each).
Per core, everything is computed in a transposed layout (partition = context
token for scores) so probs never need a transpose: U^T = V'^T @ P^T with a
ones-column in V' producing the softmax denominators for free, and the
instrument mask is applied by zeroing masked tokens' V' rows.

Key implementation notes:
  - all matmul operands are fp16 (1 cyc/row on the PE; f32 PSUM accum),
    probs are fp16 with a constant exp shift of -14 (cancels in U/Z),
  - embeddings are pre-added into context on the host; K-bias cancels in
    softmax; V-bias folds into the output bias (bv @ Wo.T) on the host,
  - inputs split across the two HWDGE queues (qt/out on sync; weights on
    scalar) + gpsimd SWDGE (context slabs), all whole-tensor transfers,
  - K^T / V' tiles are persistent (all 16 token tiles) so slab ns+1's
    projections overlap slab ns's attention without WAR stalls,
  - one 2048-wide exp ACT per j-iteration ([128, hi, half, 512] PSUM tile),
  - reciprocal_approx_fast requires a base-partition-0 operand: Z must be
    copied out of U's partition 64 first (hw misreads it otherwise).
"""

import sys

sys.path.insert(0, "/opt/trn_rl_repo")

import numpy as np

import concourse.bacc as bacc
import concourse.tile as tile
from concourse import mybir
from concourse.bass_utils import run_bass_kernel_spmd

B, T, N_CTX, H = 4, 512, 2048, 1024
NUM_HEADS, NUM_INSTRUMENTS, MAX_BARS = 16, 16, 8
HEAD_DIM = H // NUM_HEADS  # 64
HG = 2  # head groups (cores per batch)
CH = H // HG  # 512 channels per core
NH_G = NUM_HEADS // HG  # 8 heads per core
P = 128
F32 = mybir.dt.float32
FP16 = mybir.dt.float16
DT = FP16  # all matmul operands
SHIFT = -14.0  # constant exp shift: scores max ~22 -> exp <= e^8 fits fp16

KC = H // P  # 8 contraction chunks for K/V/Q projections
PT_CH = CH // P  # 4 partition tiles of channels
NS = N_CTX // 512  # 4 context slabs of 512 tokens
NT = N_CTX // P  # 16 context tiles of 128 tokens
TT = T // P  # 4 tiles of query tokens

_compiled = None


def _build():
    nc = bacc.Bacc("TRN2", target_bir_lowering=False, debug=False, num_devices=8)

    qT_d = nc.dram_tensor("qT", [P, KC, T], DT, kind="ExternalInput")
    wq_d = nc.dram_tensor("wqT", [P, KC, CH], DT, kind="ExternalInput")
    wk_d = nc.dram_tensor("wkT", [P, KC, CH], DT, kind="ExternalInput")
    wv_d = nc.dram_tensor("wvT", [P, KC, CH], DT, kind="ExternalInput")
    ctx_d = nc.dram_tensor("ctxT", [NS, P, KC, 512], DT, kind="ExternalInput")
    wo_d = nc.dram_tensor("woT", [P, PT_CH, H], DT, kind="ExternalInput")
    mb_d = nc.dram_tensor("mb", [P, NT], F32, kind="ExternalInput")
    bqe_d = nc.dram_tensor("bqe", [P, PT_CH], F32, kind="ExternalInput")
    out_d = nc.dram_tensor("out", [T, H], F32, kind="ExternalOutput")

    with tile.TileContext(nc) as tc:
        with (
            nc.allow_low_precision(reason="16-bit matmul operands; accum stays f32"),
            tc.tile_pool(name="persist", bufs=1) as pers,
        ):
            # sync HWDGE: bqe, qt, mask; scalar HWDGE: all weights;
            # gpsimd SWDGE: context slabs (in the loop below)
            bqe = pers.tile([P, PT_CH], F32, name="bqe")
            nc.sync.dma_start(bqe[:], bqe_d.ap())
            qt = pers.tile([P, KC, T], DT, name="qt_in")
            nc.sync.dma_start(qt[:], qT_d.ap())
            mb = pers.tile([P, NT], F32, name="mb")
            nc.sync.dma_start(mb[:], mb_d.ap())
            wq = pers.tile([P, KC, CH], DT, name="wq")
            nc.scalar.dma_start(wq[:], wq_d.ap())
            wk = pers.tile([P, KC, CH], DT, name="wk")
            nc.scalar.dma_start(wk[:], wk_d.ap())
            wv = pers.tile([P, KC, CH], DT, name="wv")
            nc.scalar.dma_start(wv[:], wv_d.ap())
            wo = pers.tile([P, PT_CH, H], DT, name="wo")
            nc.scalar.dma_start(wo[:], wo_d.ap())

            ones8 = pers.tile([P, NH_G], F32, name="ones8")
            nc.vector.memset(ones8[:], 1.0)
            shiftb = pers.tile([P, 1], F32, name="shiftb")
            nc.vector.memset(shiftb[:], SHIFT)
            oneshd_f = pers.tile([1, HEAD_DIM], F32, name="oneshd_f")
            nc.vector.memset(oneshd_f[:], 1.0)

            QT = [pers.tile([P, T], DT, name=f"qt{p}") for p in range(PT_CH)]
            OT = [pers.tile([P, T], DT, name=f"ot{p}") for p in range(PT_CH)]
            U = [pers.tile([HEAD_DIM + 1, T], F32, name=f"u{h}") for h in range(NH_G)]
            # persistent K^T / V' tiles for all 16 token tiles
            KT = [pers.tile([P, 512], DT, name=f"kt{i}") for i in range(NS * PT_CH)]
            VT = [pers.tile([P, NH_G, HEAD_DIM + 1], DT, name=f"vt{i}") for i in range(NT)]

            # ---- Q projection ----
            with tc.tile_pool(name="qps", bufs=1, space="PSUM") as qps:
                for p in range(PT_CH):
                    ps = qps.tile([P, 512], F32, name="ps_q", bufs=2)
                    for k in range(KC):
                        nc.tensor.matmul(
                            ps[:],
                            wq[:, k, p * P : (p + 1) * P],
                            qt[:, k, :],
                            start=(k == 0),
                            stop=(k == KC - 1),
                        )
                    nc.vector.tensor_scalar_add(QT[p][:], ps[:], bqe[:, p : p + 1])

            # ---- fused K/V projection + attention, one 512-token slab at a time ----
            with (
                tc.tile_pool(name="slab", bufs=3) as slabp,
                tc.tile_pool(name="ptp", bufs=2) as ptp,
                tc.tile_pool(name="kvps", bufs=2, space="PSUM") as kvps,
                tc.tile_pool(name="sps", bufs=1, space="PSUM") as sps,
                tc.tile_pool(name="ups", bufs=2, space="PSUM") as ups,
            ):
                for ns in range(NS):
                    slab = slabp.tile([P, KC, 512], DT, name="slab")
                    nc.gpsimd.dma_start(slab[:], ctx_d.ap()[ns])
                    # K^T columns for this slab: 4 partition tiles of channels
                    for p in range(PT_CH):
                        ps = kvps.tile([P, 512], F32, name="ps_kv")
                        for k in range(KC):
                            nc.tensor.matmul(
                                ps[:],
                                wk[:, k, p * P : (p + 1) * P],
                                slab[:, k, :],
                                start=(k == 0), stop=(k == KC - 1),
                            )
                        nc.vector.tensor_copy(KT[ns * PT_CH + p][:], ps[:])
                    # V' tiles (with masked rows zeroed, ones column for Z)
                    for s4 in range(4):
                        i = ns * 4 + s4
                        psv = kvps.tile([P, 512], F32, name="ps_kv")
                        for k in range(KC):
                            nc.tensor.matmul(
                                psv[:],
                                slab[:, k, s4 * P : (s4 + 1) * P],
                                wv[:, k, :],
                                start=(k == 0), stop=(k == KC - 1),
                            )
                        nc.vector.tensor_scalar_mul(
                            VT[i][:, :, :HEAD_DIM],
                            psv[:].rearrange("p (h d) -> p h d", d=HEAD_DIM),
                            mb[:, i : i + 1],
                        )
                        nc.vector.tensor_scalar_mul(
                            VT[i][:, :, HEAD_DIM], ones8[:], mb[:, i : i + 1]
                        )
                    # attention: scores (hi pair on disjoint 64-row PE groups)
                    # -> one 2048-wide exp -> U accumulation
                    for hp in range(NH_G // 2):
                        p = hp
                        kt = KT[ns * PT_CH + p]
                        psus = [ups.tile([HEAD_DIM + 1, 512], F32, name="ps_u") for _ in range(2)]
                        for j in range(2):
                            pss = sps.tile([P, 2, 2, 512], F32, name="ps_s")
                            pts = ptp.tile([P, 2, 2, 512], DT, name="pt")
                            for half in range(2):
                                s4 = 2 * j + half
                                for hi in range(2):
                                    d0, d1 = hi * HEAD_DIM, (hi + 1) * HEAD_DIM
                                    nc.tensor.matmul(
                                        pss[:, hi, half, :],
                                        kt[d0:d1, s4 * P : (s4 + 1) * P],
                                        QT[p][d0:d1, :],
                                        start=True, stop=True,
                                    )
                            nc.scalar.activation(
                                pts[:], pss[:], mybir.ActivationFunctionType.Exp,
                                bias=shiftb[:], scale=0.125,
                            )
                            for hi in range(2):
                                for half in range(2):
                                    s4 = 2 * j + half
                                    nc.tensor.matmul(
                                        psus[hi][:], VT[ns * 4 + s4][:, 2 * hp + hi, :],
                                        pts[:, hi, half, :],
                                        start=(j == 0 and half == 0),
                                        stop=(j == 1 and half == 1),
                                    )
                        for hi in range(2):
                            h = 2 * hp + hi
                            if ns == 0:
                                nc.vector.tensor_copy(U[h][:], psus[hi][:])
                            else:
                                nc.vector.tensor_add(U[h][:], U[h][:], psus[hi][:])

            # ---- normalization + output projection ----
            with (
                tc.tile_pool(name="nsb", bufs=4) as nsb,
                tc.tile_pool(name="ob", bufs=3) as obp,
                tc.tile_pool(name="rps", bufs=2, space="PSUM") as rps,
                tc.tile_pool(name="ops", bufs=3, space="PSUM") as ops,
            ):
                for h in range(NH_G):
                    p, hi = h // 2, h % 2
                    zst = nsb.tile([1, T], F32, name="zst")
                    nc.vector.tensor_copy(zst[:], U[h][HEAD_DIM : HEAD_DIM + 1, :])
                    rf = nsb.tile([1, T], F32, name="rf")
                    nc.vector.reciprocal_approx_fast(rf[:], zst[:])
                    psr = rps.tile([HEAD_DIM, 512], F32, name="ps_r")
                    nc.tensor.matmul(
                        psr[:], oneshd_f[:], rf[:], start=True, stop=True,
                    )
                    d0, d1 = hi * HEAD_DIM, (hi + 1) * HEAD_DIM
                    nc.vector.tensor_tensor(
                        OT[p][d0:d1, :], U[h][:HEAD_DIM, :], psr[:],
                        op=mybir.AluOpType.mult,
                    )

                for tt in range(TT):
                    for o in range(2):
                        pso = ops.tile([P, 512], F32, name="ps_o")
                        for p in range(PT_CH):
                            nc.tensor.matmul(
                                pso[:],
                                OT[p][:, tt * P : (tt + 1) * P],
                                wo[:, p, o * 512 : (o + 1) * 512],
                                start=(p == 0), stop=(p == PT_CH - 1),
                            )
                        ob = obp.tile([P, 512], F32, name="ob")
                        nc.scalar.copy(ob[:], pso[:])
                        nc.sync.dma_start(
                            out_d.ap()[tt * P : (tt + 1) * P, o * 512 : (o + 1) * 512],
                            ob[:],
                        )

    nc.compile()
    return nc


def _prep_inputs(query, context, instrument_ids, current_instrument_id, bar_offsets,
                 Wq, bq, Wk, bk, Wv, bv, Wo, bo, inst_emb, bar_emb):
    f32, f16 = np.float32, np.float16
    query = np.asarray(query, f32)
    context = np.asarray(context, f32)
    inst = np.asarray(instrument_ids).astype(np.int64)
    bars = np.clip(np.asarray(bar_offsets).astype(np.int64), 0, MAX_BARS - 1)
    cur = min(max(int(np.asarray(current_instrument_id)), 0), NUM_INSTRUMENTS - 1)
    Wq, Wk, Wv, Wo = (np.asarray(w, f32) for w in (Wq, Wk, Wv, Wo))
    bq, bv, bo = (np.asarray(b, f32) for b in (bq, bv, bo))
    inst_emb = np.asarray(inst_emb, f32)
    bar_emb = np.asarray(bar_emb, f32)

    ctx = context + inst_emb[inst] + bar_emb[bars]        # (B,N,H) host pre-add
    qh = query + inst_emb[cur][None, None, :]
    bq_eff = bq + 0.0

    def perm(x):  # (H, F) -> [128, KC, F] contiguous
        Hd, F = x.shape
        return np.ascontiguousarray(x.reshape(KC, P, F).transpose(1, 0, 2).astype(f16))

    WqT, WkT, WvT, WoT = Wq.T, Wk.T, Wv.T, Wo.T

    in_maps = []
    for b in range(B):
        qT = perm(np.ascontiguousarray(qh[b].T))          # [128, 8, 512]
        ctxT = ctx[b].T                                   # (H, N)
        # [NS, 128, KC, 512]
        ctx_p = np.ascontiguousarray(
            ctxT.reshape(KC, P, NS, 512).transpose(2, 1, 0, 3).astype(f16))
        mbv = np.where(inst[b] == cur, 0.0, 1.0).astype(f32)
        mbt = np.ascontiguousarray(mbv.reshape(NT, P).T)  # (128, NT)
        for g in range(HG):
            sl = slice(g * CH, (g + 1) * CH)
            woT = np.ascontiguousarray(
                WoT[sl, :].reshape(PT_CH, P, H).transpose(1, 0, 2).astype(f16))
            in_maps.append({
                "qT": qT,
                "ctxT": ctx_p,
                "wqT": perm(WqT[:, sl]),
                "wkT": perm(WkT[:, sl]),
                "wvT": perm(WvT[:, sl]),
                "woT": woT,
                "mb": mbt,
                "bqe": np.ascontiguousarray(bq_eff[sl].reshape(PT_CH, P).T),
            })
    return in_maps, bo + bv @ Wo.T


def kernel(**inputs) -> np.ndarray:
    global _compiled
    if _compiled is None:
        _compiled = _build()
    in_maps, bo_eff = _prep_inputs(**inputs)
    res = run_bass_kernel_spmd(_compiled, in_maps, list(range(B * HG))).results
    out = np.empty((B, T, H), np.float32)
    for b in range(B):
        out[b] = res[b * HG]["out"] + res[b * HG + 1]["out"] + bo_eff
    return out
